# revision 7
# baseline (speedup 1.0000x reference)
"""Size-weighted focal loss on 8 Trainium2 NeuronCores — v2.

Math (per element, x = logit, t in {0,1}):
  w  = x*(1-2t)          so (1-pt) = sigmoid(w)
  N  = softplus(-w)      = ln(1 + e^{-w})
  L  = -log(pt) = softplus(w) = N + w
  s2 = sigmoid(w)^2      = e^{-2N}
  a  = 0.75 - 0.5*t      (alpha_t with ALPHA=0.25)
  elem = a * s2 * L

Device formulation (per core, 8 samples of [128,2048], bf16 intermediates):
  w   = bf16 sign-flip of x: top-16-bit view of x XOR (t16 << 15)   [DVE 1x]
  E   = exp(-w)                                                     [ACT]
  N   = ln(E + 1)                                                   [ACT]
  s2p = exp(-2N + ln 0.5) = 0.5*s2                                  [ACT]
  tf  = bf16(t)                                                     [GPSIMD]
  F'  = (tf - 1.5) * s2p = -(a*s2)                                  [DVE 2x]
  PE group1: psum[128,256] += F'^T @ [N | w]  (16 chunks)
  PE group2: psum[128,1]   += tf^T @ ones     (16 chunks, fg partials)
  diag extract with mask M[i,i] = M[i,128+i] = -1:
    Scol[:,b] = sum_j ps*mask = sum a*s2*(N+w) partials per partition-slot
  All three activations resolve to the natural_log_exp_and_others table
  set (see _patch_act_tables) -> one ACT_TABLE_LOAD per kernel.

Host combines per-sample sums: mean_b( (S_b/HW) * sw(fg_b) ).
"""

import numpy as np
from contextlib import ExitStack

P = 128
B_PER_CORE = 8
N_CORES = 8
H = 512
W = 512
HW = H * W                 # 262144
FREE = HW // P             # 2048
NCHUNK = FREE // P         # 16
LN_HALF = -0.6931471805599453

_GLOBAL = {}


def _patch_act_tables():
    """Steer every Exp/Ln activation to the one table set containing both
    (natural_log_exp_and_others), instead of the greedy first-match which
    alternates exp_and_others/natural_log and reloads tables per sample.
    Set order/indices are preserved; only membership is masked."""
    import concourse.bacc as bacc_mod
    import concourse.mybir as mybir
    from concourse.hw_specs import get_activation_tables as _orig

    def _patched(arch):
        A = mybir.ActivationFunctionType
        out = {}
        for name, fns in _orig(arch).items():
            if name != "natural_log_exp_and_others":
                fns = fns - {A.Exp, A.Ln}
            out[name] = fns
        return out

    bacc_mod.get_activation_tables = _patched


def _build():
    import concourse.bacc as bacc
    import concourse.tile as tile
    import concourse.mybir as mybir

    _patch_act_tables()

    f32 = mybir.dt.float32
    i32 = mybir.dt.int32
    bf16 = mybir.dt.bfloat16
    u16 = mybir.dt.uint16
    Alu = mybir.AluOpType
    Act = mybir.ActivationFunctionType

    nc = bacc.Bacc("TRN2", target_bir_lowering=False, debug=False,
                   num_devices=N_CORES)

    pred_in = nc.dram_tensor("pred", (B_PER_CORE, H, W), f32, kind="ExternalInput")
    targ_in = nc.dram_tensor("target", (B_PER_CORE, H, W), i32, kind="ExternalInput")
    mask_in = nc.dram_tensor("mask", (P, 2 * P), f32, kind="ExternalInput")
    out_t = nc.dram_tensor("out", (B_PER_CORE, 2), f32, kind="ExternalOutput")

    # [b, 512, 512] -> [b, 128, 2048]; partition p holds contiguous 2048 elems
    x_v = pred_in.ap().rearrange("b (p q) w -> b p (q w)", p=P)
    t_v = targ_in.ap().rearrange("b (p q) w -> b p (q w)", p=P)

    with ExitStack() as ctx:
        tc = ctx.enter_context(tile.TileContext(nc))
        singles = ctx.enter_context(tc.tile_pool(name="singles", bufs=1))
        io = ctx.enter_context(tc.tile_pool(name="io", bufs=3))
        work = ctx.enter_context(tc.tile_pool(name="work", bufs=2))
        psum = ctx.enter_context(tc.tile_pool(name="psum", bufs=4, space="PSUM"))
        psumB = ctx.enter_context(tc.tile_pool(name="psumB", bufs=3, space="PSUM"))
        psum_fin = ctx.enter_context(tc.tile_pool(name="psum_fin", bufs=1, space="PSUM"))

        mask_t = singles.tile([P, 2 * P], f32)
        nc.sync.dma_start(out=mask_t[:], in_=mask_in.ap())
        ones_b = singles.tile([P, 1], bf16)
        nc.vector.memset(ones_b[:], 1.0)
        ones_f = singles.tile([P, 1], f32)
        nc.vector.memset(ones_f[:], 1.0)
        lnhalf_t = singles.tile([P, 1], f32)
        nc.vector.memset(lnhalf_t[:], LN_HALF)
        shift15_t = singles.tile([P, 1], u16)
        nc.vector.memset(shift15_t[:], 15)
        Scol = singles.tile([P, B_PER_CORE], f32)   # per-partition loss partials
        Acol = singles.tile([P, B_PER_CORE], f32)   # per-partition fg partials

        for b in range(B_PER_CORE):
            xt = io.tile([P, FREE], f32, tag="xt")
            tt = io.tile([P, FREE], i32, tag="tt")
            nc.sync.dma_start(out=xt[:], in_=x_v[b])
            nc.sync.dma_start(out=tt[:], in_=t_v[b])

            # nw holds both PE rhs blocks: [:,0,:] = N, [:,1,:] = w
            nw = work.tile([P, 2, FREE], bf16, tag="nw")

            # w = bf16(x) with sign flipped where t==1, in one 1x DVE op:
            # (t16 << 15) XOR hi16(x).  u16 views keep the ALU integer-only.
            x_hi = xt[:].bitcast(u16).rearrange("p (q two) -> p q two", two=2)[:, :, 1]
            t_lo = tt[:].bitcast(u16).rearrange("p (q two) -> p q two", two=2)[:, :, 0]
            w_u16 = nw[:, 1, :].bitcast(u16)
            nc.vector.scalar_tensor_tensor(
                out=w_u16, in0=t_lo, scalar=shift15_t[:], in1=x_hi,
                op0=Alu.logical_shift_left, op1=Alu.bitwise_xor)

            Et = work.tile([P, FREE], bf16, tag="Et")
            nc.scalar.activation(Et[:], nw[:, 1, :], Act.Exp, scale=-1.0)
            nc.scalar.activation(nw[:, 0, :], Et[:], Act.Ln, bias=1.0)
            s2p = work.tile([P, FREE], bf16, tag="s2p")
            nc.scalar.activation(s2p[:], nw[:, 0, :], Act.Exp, scale=-2.0,
                                 bias=lnhalf_t[:])

            # tf = bf16(t) on GPSIMD (frees DVE cycles)
            tf = work.tile([P, FREE], bf16, tag="tf")
            nc.gpsimd.tensor_copy(tf[:], tt[:])

            # F' = (tf - 1.5) * 0.5*s2 = -(0.75-0.5t)*s2 = -a*s2
            Ft = work.tile([P, FREE], bf16, tag="Ft")
            nc.vector.scalar_tensor_tensor(
                out=Ft[:], in0=tf[:], scalar=1.5, in1=s2p[:],
                op0=Alu.subtract, op1=Alu.mult)

            ps = psum.tile([P, 2 * P], f32)
            for c in range(NCHUNK):
                sl = slice(c * P, (c + 1) * P)
                # psum cols 0:128 = F'^T N, cols 128:256 = F'^T w
                nc.tensor.matmul(ps[:], Ft[:, sl], nw[:, :, sl],
                                 start=(c == 0), stop=(c == NCHUNK - 1))
            psB = psumB.tile([P, 1], f32)
            for c in range(NCHUNK):
                sl = slice(c * P, (c + 1) * P)
                nc.tensor.matmul(psB[:], tf[:, sl], ones_b[:],
                                 start=(c == 0), stop=(c == NCHUNK - 1))

            scr = work.tile([P, 2 * P], f32, tag="scr")
            # Scol[:,b] = sum_j ps[:,j] * mask[:,j]  (diag picks -N, -w blocks)
            nc.vector.scalar_tensor_tensor(
                out=scr[:], in0=ps[:], scalar=0.0, in1=mask_t[:],
                op0=Alu.add, op1=Alu.mult,
                accum_out=Scol[:, b:b + 1])
            nc.vector.tensor_copy(Acol[:, b:b + 1], psB[:])

        # Final partition reduction via ones-matmul: [128,8]^T @ [128,1] -> [8,1]
        fin = psum_fin.tile([B_PER_CORE, 2], f32)
        nc.tensor.matmul(fin[:, 0:1], Scol[:], ones_f[:], start=True, stop=True)
        nc.tensor.matmul(fin[:, 1:2], Acol[:], ones_f[:], start=True, stop=True)
        out_sb = singles.tile([B_PER_CORE, 2], f32)
        nc.vector.tensor_copy(out_sb[:], fin[:])
        nc.sync.dma_start(out=out_t.ap(), in_=out_sb[:])

    nc.compile()
    return nc


def _get_nc():
    if "nc" not in _GLOBAL:
        _GLOBAL["nc"] = _build()
    return _GLOBAL["nc"]


def _mask_np():
    m = np.zeros((P, 2 * P), dtype=np.float32)
    idx = np.arange(P)
    m[idx, idx] = -1.0          # -(F' * N) = F * N
    m[idx, P + idx] = -1.0      # -(F' * w) = F * w
    return m


GAMMA = 2.0
ALPHA = 0.25
SIZE_POWER = 0.5


def kernel(pred: np.ndarray, target: np.ndarray) -> np.ndarray:
    from concourse import bass_utils

    nc = _get_nc()
    pred = np.ascontiguousarray(np.asarray(pred, dtype=np.float32))
    target = np.ascontiguousarray(np.asarray(target, dtype=np.int32))
    mask = _mask_np()

    in_maps = []
    for i in range(N_CORES):
        sl = slice(i * B_PER_CORE, (i + 1) * B_PER_CORE)
        in_maps.append({
            "pred": np.ascontiguousarray(pred[sl, 0]),
            "target": np.ascontiguousarray(target[sl]),
            "mask": mask,
        })

    res = bass_utils.run_bass_kernel_spmd(
        nc, in_maps, core_ids=list(range(N_CORES)),
        trace=bool(_GLOBAL.get("trace", False)),
        **_GLOBAL.get("run_kwargs", {}),
    )
    _GLOBAL["last_results"] = res

    outs = np.concatenate([r["out"] for r in res.results], axis=0)  # [64, 2]
    S = outs[:, 0].astype(np.float64)          # per-sample sum(a*s2*L)
    fg = np.rint(outs[:, 1].astype(np.float64))  # per-sample foreground count
    sw = np.where(fg > 0,
                  np.minimum(100.0 / np.power(np.maximum(fg, 1.0), SIZE_POWER), 10.0),
                  1.0)
    per_sample = (S / HW) * sw
    return np.float32(per_sample.mean())


# revision 17
# speedup vs baseline: 1.4017x; 1.4017x over previous
"""Size-weighted focal loss on 8 Trainium2 NeuronCores — v2.

Math (per element, x = logit, t in {0,1}):
  w  = x*(1-2t)          so (1-pt) = sigmoid(w)
  N  = softplus(-w)      = ln(1 + e^{-w})
  L  = -log(pt) = softplus(w) = N + w
  s2 = sigmoid(w)^2      = e^{-2N}
  a  = 0.75 - 0.5*t      (alpha_t with ALPHA=0.25)
  elem = a * s2 * L

Device formulation (per core, 8 samples of [128,2048], bf16 intermediates):
  w   = bf16 sign-flip of x: top-16-bit view of x XOR (t16 << 15)   [DVE 1x]
  E   = exp(-w)                                                     [ACT]
  N   = ln(E + 1)                                                   [ACT]
  s2p = exp(-2N + ln 0.5) = 0.5*s2                                  [ACT]
  tf  = bf16(t)                                                     [GPSIMD]
  F'  = (tf - 1.5) * s2p = -(a*s2)                                  [DVE 2x]
  PE group1: psum[128,256] += F'^T @ [N | w]  (16 chunks)
  PE group2: psum[128,1]   += tf^T @ ones     (16 chunks, fg partials)
  diag extract with mask M[i,i] = M[i,128+i] = -1:
    Scol[:,b] = sum_j ps*mask = sum a*s2*(N+w) partials per partition-slot
  All three activations resolve to the natural_log_exp_and_others table
  set (see _patch_act_tables) -> one ACT_TABLE_LOAD per kernel.

Host combines per-sample sums: mean_b( (S_b/HW) * sw(fg_b) ).
"""

import numpy as np
from contextlib import ExitStack

P = 128
B_PER_CORE = 8
N_CORES = 8
H = 512
W = 512
HW = H * W                 # 262144
FREE = HW // P             # 2048
NCHUNK = FREE // P         # 16
LN_HALF = -0.6931471805599453

_GLOBAL = {}


def _patch_act_tables():
    """Steer every Exp/Ln activation to the one table set containing both
    (natural_log_exp_and_others), instead of the greedy first-match which
    alternates exp_and_others/natural_log and reloads tables per sample.
    Set order/indices are preserved; only membership is masked."""
    import concourse.bacc as bacc_mod
    import concourse.mybir as mybir
    from concourse.hw_specs import get_activation_tables as _orig

    def _patched(arch):
        A = mybir.ActivationFunctionType
        out = {}
        for name, fns in _orig(arch).items():
            if name != "natural_log_exp_and_others":
                fns = fns - {A.Exp, A.Ln}
            out[name] = fns
        return out

    bacc_mod.get_activation_tables = _patched


def _build():
    import concourse.bacc as bacc
    import concourse.tile as tile
    import concourse.mybir as mybir

    _patch_act_tables()

    f32 = mybir.dt.float32
    i32 = mybir.dt.int32
    bf16 = mybir.dt.bfloat16
    u16 = mybir.dt.uint16
    Alu = mybir.AluOpType
    Act = mybir.ActivationFunctionType

    nc = bacc.Bacc("TRN2", target_bir_lowering=False, debug=False,
                   num_devices=N_CORES)

    pred_in = nc.dram_tensor("pred", (B_PER_CORE, H, W), f32, kind="ExternalInput")
    targ_in = nc.dram_tensor("target", (B_PER_CORE, H, W), i32, kind="ExternalInput")
    mask_in = nc.dram_tensor("mask", (P, 2 * P), f32, kind="ExternalInput")
    out_t = nc.dram_tensor("out", (B_PER_CORE, 1), f32, kind="ExternalOutput")

    # [b, 512, 512] -> [b, 128, 2048]; partition p holds contiguous 2048 elems
    x_v = pred_in.ap().rearrange("b (p q) w -> b p (q w)", p=P)
    t_v = targ_in.ap().rearrange("b (p q) w -> b p (q w)", p=P)

    with ExitStack() as ctx:
        tc = ctx.enter_context(tile.TileContext(nc))
        singles = ctx.enter_context(tc.tile_pool(name="singles", bufs=1))
        io = ctx.enter_context(tc.tile_pool(name="io", bufs=3))
        work = ctx.enter_context(tc.tile_pool(name="work", bufs=2))
        psum = ctx.enter_context(tc.tile_pool(name="psum", bufs=4, space="PSUM"))
        psum_fin = ctx.enter_context(tc.tile_pool(name="psum_fin", bufs=1, space="PSUM"))

        mask_t = singles.tile([P, 2 * P], f32)
        nc.sync.dma_start(out=mask_t[:], in_=mask_in.ap())
        ones_f = singles.tile([P, 1], f32)
        nc.vector.memset(ones_f[:], 1.0)
        shift31_t = singles.tile([P, 1], i32)
        nc.vector.memset(shift31_t[:], 31)
        Scol = singles.tile([P, B_PER_CORE], f32)   # per-partition loss partials

        for b in range(B_PER_CORE):
            xt = io.tile([P, FREE], f32, tag="xt")
            tt = io.tile([P, FREE], i32, tag="tt")
            nc.sync.dma_start(out=xt[:], in_=x_v[b])
            nc.sync.dma_start(out=tt[:], in_=t_v[b])

            # nw holds both PE rhs blocks: [:,0,:] = N, [:,1,:] = w
            nw = work.tile([P, 2, FREE], bf16, tag="nw")

            # w = x * (1-2t) via sign-bit XOR: (t << 31) ^ bits(x), all
            # contiguous 4-byte operands (1x DVE).
            wt = work.tile([P, FREE], f32, tag="wt")
            nc.vector.scalar_tensor_tensor(
                out=wt[:].bitcast(i32), in0=tt[:], scalar=shift31_t[:],
                in1=xt[:].bitcast(i32),
                op0=Alu.logical_shift_left, op1=Alu.bitwise_xor)
            # wb = bf16(w) for the PE rhs block (2x_2p copy)
            nc.vector.tensor_copy(nw[:, 1, :], wt[:])

            Et = work.tile([P, FREE], bf16, tag="Et")
            nc.scalar.activation(Et[:], wt[:], Act.Exp, scale=-1.0)
            nc.scalar.activation(nw[:, 0, :], Et[:], Act.Ln, bias=1.0)
            s2t = work.tile([P, FREE], bf16, tag="s2t")
            nc.scalar.activation(s2t[:], nw[:, 0, :], Act.Exp, scale=-2.0)

            # at = 0.75 - 0.5*t (= alpha_t); fg is counted host-side
            at = work.tile([P, FREE], bf16, tag="at")
            nc.vector.tensor_scalar(at[:], tt[:], -0.5, 0.75,
                                    Alu.mult, Alu.add)
            # F = at * s2 = a * sigmoid(w)^2   (bf16 2x_1p)
            Ft = work.tile([P, FREE], bf16, tag="Ft")
            nc.vector.tensor_tensor(Ft[:], at[:], s2t[:], Alu.mult)

            ps = psum.tile([P, 2 * P], f32)
            for c in range(NCHUNK):
                sl = slice(c * P, (c + 1) * P)
                # psum cols 0:128 = F^T N, cols 128:256 = F^T w
                nc.tensor.matmul(ps[:], Ft[:, sl], nw[:, :, sl],
                                 start=(c == 0), stop=(c == NCHUNK - 1))

            scr = work.tile([P, 2 * P], f32, tag="scr")
            # Scol[:,b] = sum_j ps[:,j] * mask[:,j]  (diag picks +N, +w blocks)
            nc.vector.scalar_tensor_tensor(
                out=scr[:], in0=ps[:], scalar=0.0, in1=mask_t[:],
                op0=Alu.add, op1=Alu.mult,
                accum_out=Scol[:, b:b + 1])

        # Final partition reduction via ones-matmul: [128,8]^T @ [128,1] -> [8,1]
        fin = psum_fin.tile([B_PER_CORE, 1], f32)
        nc.tensor.matmul(fin[:, 0:1], Scol[:], ones_f[:], start=True, stop=True)
        out_sb = singles.tile([B_PER_CORE, 1], f32)
        nc.vector.tensor_copy(out_sb[:], fin[:])
        nc.sync.dma_start(out=out_t.ap(), in_=out_sb[:])

    nc.compile()
    return nc


def _get_nc():
    if "nc" not in _GLOBAL:
        _GLOBAL["nc"] = _build()
    return _GLOBAL["nc"]


def _mask_np():
    m = np.zeros((P, 2 * P), dtype=np.float32)
    idx = np.arange(P)
    m[idx, idx] = 1.0           # F * N block
    m[idx, P + idx] = 1.0       # F * w block
    return m


GAMMA = 2.0
ALPHA = 0.25
SIZE_POWER = 0.5


def kernel(pred: np.ndarray, target: np.ndarray) -> np.ndarray:
    from concourse import bass_utils

    nc = _get_nc()
    pred = np.ascontiguousarray(np.asarray(pred, dtype=np.float32))
    target = np.ascontiguousarray(np.asarray(target, dtype=np.int32))
    mask = _mask_np()

    in_maps = []
    for i in range(N_CORES):
        sl = slice(i * B_PER_CORE, (i + 1) * B_PER_CORE)
        in_maps.append({
            "pred": np.ascontiguousarray(pred[sl, 0]),
            "target": np.ascontiguousarray(target[sl]),
            "mask": mask,
        })

    res = bass_utils.run_bass_kernel_spmd(
        nc, in_maps, core_ids=list(range(N_CORES)),
        trace=bool(_GLOBAL.get("trace", False)),
        **_GLOBAL.get("run_kwargs", {}),
    )
    _GLOBAL["last_results"] = res

    outs = np.concatenate([r["out"] for r in res.results], axis=0)  # [64, 1]
    S = outs[:, 0].astype(np.float64)          # per-sample sum(a*s2*L)
    fg = np.count_nonzero(target.reshape(target.shape[0], -1), axis=1)
    fg = fg.astype(np.float64)
    sw = np.where(fg > 0,
                  np.minimum(100.0 / np.power(np.maximum(fg, 1.0), SIZE_POWER), 10.0),
                  1.0)
    per_sample = (S / HW) * sw
    return np.float32(per_sample.mean())
